# revision 13
# baseline (speedup 1.0000x reference)
"""Multi-head attention (B=4, S=2048, D=1024, H=16, causal) on 8 Trainium2 cores.

Sharding: core c -> (batch b = c//2, head-group hg = c%2, 8 heads each).
Each core computes its 8 heads' attention for its batch element plus the
partial output projection against the corresponding 512 columns of Wo.
Host sums the two partial projections per batch element and adds bo.

Device-side layouts (prepared on host as part of sharding):
  xqT/xkT/xvT [D=1024, S=2048]  -- x.T so the contraction dim (d) sits on
                                   SBUF partitions for all projection matmuls
  wqT/wkT/wvT [1024, 512]       -- W_part.T ([d, d'])
  woT [512, 1024]               -- Wo[:, part].T ([d', dout])
  bq/bk/bv [512], tri [128,128] -- triu(ones): tri[k,q] = 1 iff k <= q

All matmuls run as float32r (full fp32 storage, reduced-precision multiply,
1 cycle/row for moving free dim >= 256). Scores are computed transposed
(S_T[k, q]) so softmax needs no on-chip transposes: exp(s/8) on ScalarE
(no max subtraction; scores are ~N(0,1) for this problem's inputs), the
softmax denominator comes from a ones-column appended to V, and the
normalization happens on the [65, q] PV accumulator where l is a single
partition row.
"""

import os
import sys

import numpy as np

for _p in ("/opt/trn_rl_repo", "/root/.axon_site/_ro/trn_rl_repo"):
    if os.path.isdir(_p):
        if _p not in sys.path:
            sys.path.insert(0, _p)
        break

import concourse.bass as bass
import concourse.bacc as bacc
import concourse.tile as tile
from concourse import mybir
from concourse import bass_utils

B, S, D, H = 4, 2048, 1024, 16
HD = D // H            # 64
NCORES = 8
HPC = 8                # heads per core
DPC = 512              # d' (head dims) per core
NPAIR = 4              # head pairs per core
KT = S // 128          # 16 k-tiles
QT = S // 512          # 4 q-tiles (512 wide)
DT = D // 128          # 8 d-tiles
JT = DPC // 128        # 4 d'-tiles

F32 = mybir.dt.float32
F32R = mybir.dt.float32r

_NC_CACHE = {}


def _emit(tc, debug=False):
    nc = tc.nc

    xqT = nc.dram_tensor("xqT", [D, S], F32R, kind="ExternalInput").ap()
    xkT = nc.dram_tensor("xkT", [D, S], F32R, kind="ExternalInput").ap()
    xvT = nc.dram_tensor("xvT", [D, S], F32R, kind="ExternalInput").ap()
    wqT = nc.dram_tensor("wqT", [D, DPC], F32R, kind="ExternalInput").ap()
    wkT = nc.dram_tensor("wkT", [D, DPC], F32R, kind="ExternalInput").ap()
    wvT = nc.dram_tensor("wvT", [D, DPC], F32R, kind="ExternalInput").ap()
    woT = nc.dram_tensor("woT", [DPC, D], F32R, kind="ExternalInput").ap()
    bqd = nc.dram_tensor("bq", [DPC], F32, kind="ExternalInput").ap()
    bkd = nc.dram_tensor("bk", [DPC], F32, kind="ExternalInput").ap()
    bvd = nc.dram_tensor("bv", [DPC], F32, kind="ExternalInput").ap()
    trid = nc.dram_tensor("tri", [128, 128], F32R, kind="ExternalInput").ap()
    onesd = nc.dram_tensor("ones", [KT, HPC], F32R, kind="ExternalInput").ap()
    z = nc.dram_tensor("z", [S, D], F32, kind="ExternalOutput").ap()
    dbg = {}
    if debug:
        dbg["qT"] = nc.dram_tensor("dbg_qT", [128, NPAIR, S], F32, kind="ExternalOutput").ap()
        dbg["kT"] = nc.dram_tensor("dbg_kT", [128, NPAIR, S], F32, kind="ExternalOutput").ap()
        dbg["v"] = nc.dram_tensor("dbg_v", [128, KT, HPC, 65], F32, kind="ExternalOutput").ap()
        dbg["p0"] = nc.dram_tensor("dbg_p0", [128, 1024], F32, kind="ExternalOutput").ap()
        dbg["p1"] = nc.dram_tensor("dbg_p1", [128, 1024], F32, kind="ExternalOutput").ap()
        dbg["pv0"] = nc.dram_tensor("dbg_pv0", [65, 512], F32, kind="ExternalOutput").ap()
        dbg["rlb"] = nc.dram_tensor("dbg_rlb", [64, 512], F32, kind="ExternalOutput").ap()
        dbg["ont"] = nc.dram_tensor("dbg_ont", [64, HPC, 512], F32, kind="ExternalOutput").ap()

    from contextlib import ExitStack

    with ExitStack() as stack:
        singles = stack.enter_context(tc.tile_pool(name="singles", bufs=1))
        qkv = stack.enter_context(tc.tile_pool(name="qkv", bufs=1))

        tri_sb = singles.tile([128, 128], F32R)
        nc.sync.dma_start(out=tri_sb, in_=trid)
        bvb = singles.tile([128, DPC], F32)
        nc.gpsimd.dma_start(out=bvb, in_=bvd.partition_broadcast(128))
        bq_sb = singles.tile([128, JT], F32)
        nc.sync.dma_start(out=bq_sb, in_=bqd.rearrange("(j p) -> p j", p=128))
        bk_sb = singles.tile([128, JT], F32)
        nc.sync.dma_start(out=bk_sb, in_=bkd.rearrange("(j p) -> p j", p=128))

        qT_sb = qkv.tile([128, NPAIR, S], F32R)   # [d'-in-pair, pair, q]
        kT_sb = qkv.tile([128, NPAIR, S], F32R)
        # V augmented per head: cols 0:64 = V_h, col 64 = ones (softmax denom)
        v_sb = qkv.tile([128, KT, HPC, 65], F32R)

        # ones column: v_ones = tri_view * 0 + 1 (memset can't write f32r)
        tri_view = tri_sb.rearrange("p (a b) -> p a b", a=KT).unsqueeze(3)
        nc.vector.tensor_scalar(
            v_sb[:, :, :, 64:65],
            tri_view,
            0.0,
            1.0,
            mybir.AluOpType.mult,
            mybir.AluOpType.add,
        )

        # ---------------- Phase 1a: Q and K projections -------------------
        with (
            tc.tile_pool(name="wqk", bufs=1) as wqk_pool,
            tc.tile_pool(name="xchunk", bufs=16) as xchunk_pool,
            tc.tile_pool(name="proj_ps", bufs=4, space="PSUM") as proj_ps,
        ):
            wq_sb = wqk_pool.tile([128, DT, DPC], F32R)
            nc.sync.dma_start(out=wq_sb, in_=wqT.rearrange("(dt p) c -> p dt c", p=128))
            wk_sb = wqk_pool.tile([128, DT, DPC], F32R)
            nc.sync.dma_start(out=wk_sb, in_=wkT.rearrange("(dt p) c -> p dt c", p=128))

            for w_sb, xT, dst_sb, b_sb in (
                (wq_sb, xqT, qT_sb, bq_sb),
                (wk_sb, xkT, kT_sb, bk_sb),
            ):
                for t in range(QT):
                    chunks = []
                    for dt in range(DT):
                        ch = xchunk_pool.tile([128, 512], F32R, tag="xch")
                        nc.sync.dma_start(
                            out=ch,
                            in_=xT[128 * dt : 128 * (dt + 1), 512 * t : 512 * (t + 1)],
                        )
                        chunks.append(ch)
                    for j in range(JT):
                        ps = proj_ps.tile([128, 512], F32)
                        for dt in range(DT):
                            nc.tensor.matmul(
                                ps,
                                w_sb[:, dt, 128 * j : 128 * (j + 1)],
                                chunks[dt],
                                start=(dt == 0),
                                stop=(dt == DT - 1),
                            )
                        nc.vector.tensor_scalar_add(
                            dst_sb[:, j, 512 * t : 512 * (t + 1)], ps, b_sb[:, j : j + 1]
                        )

        # ---------------- Phase 1b: V projection --------------------------
        with (
            tc.tile_pool(name="wv", bufs=1) as wv_pool,
            tc.tile_pool(name="xvchunk", bufs=16) as xvchunk_pool,
            tc.tile_pool(name="projv_ps", bufs=4, space="PSUM") as projv_ps,
        ):
            wv_sb = wv_pool.tile([128, DT, DPC], F32R)
            nc.sync.dma_start(out=wv_sb, in_=wvT.rearrange("(dt p) c -> p dt c", p=128))

            for ktg in range(4):  # groups of 4 k-tiles
                vchunks = []
                for dt in range(DT):
                    ch = xvchunk_pool.tile([128, 512], F32R, tag="xvch")
                    nc.sync.dma_start(
                        out=ch,
                        in_=xvT[
                            128 * dt : 128 * (dt + 1), 512 * ktg : 512 * (ktg + 1)
                        ],
                    )
                    vchunks.append(ch)
                for ksub in range(4):
                    kt = 4 * ktg + ksub
                    ps = projv_ps.tile([128, 512], F32)
                    for dt in range(DT):
                        nc.tensor.matmul(
                            ps,
                            vchunks[dt][:, 128 * ksub : 128 * (ksub + 1)],
                            wv_sb[:, dt, :],
                            start=(dt == 0),
                            stop=(dt == DT - 1),
                        )
                    # scatter heads into the augmented layout (+ bias)
                    ps4 = ps.rearrange("p (h c) -> p h c", h=HPC)
                    bv4 = bvb.rearrange("p (h c) -> p h c", h=HPC)
                    nc.vector.tensor_add(v_sb[:, kt, :, 0:64], ps4, bv4)

        if debug:
            nc.sync.dma_start(out=dbg["qT"], in_=qT_sb.bitcast(F32))
            nc.sync.dma_start(out=dbg["kT"], in_=kT_sb.bitcast(F32))
            nc.sync.dma_start(out=dbg["v"], in_=v_sb.bitcast(F32))

        # ---------------- Phase 2: attention + output projection ----------
        with (
            tc.tile_pool(name="wo", bufs=1) as wo_pool,
            tc.tile_pool(name="p_sb", bufs=4) as p_pool,
            tc.tile_pool(name="o_nt", bufs=2) as o_pool,
            tc.tile_pool(name="z_sb", bufs=2) as z_pool,
            tc.tile_pool(name="rl", bufs=2) as rl_pool,
            tc.tile_pool(name="rlb", bufs=4) as rlb_pool,
            tc.tile_pool(name="score_ps", bufs=2, space="PSUM") as score_ps,
            tc.tile_pool(name="pv_ps", bufs=2, space="PSUM") as pv_ps,
            tc.tile_pool(name="z_ps", bufs=2, space="PSUM") as z_ps,
        ):
            # per-head Wo.T rows, all at base partition 0: [64, head, dout]
            woT_sb = wo_pool.tile([64, HPC, D], F32R)
            nc.sync.dma_start(
                out=woT_sb, in_=woT.rearrange("(h p) c -> p h c", p=64)
            )

            for t in range(QT):
                nki = 4 * (t + 1)
                qsl = slice(512 * t, 512 * (t + 1))
                o_nt = o_pool.tile([64, HPC, 512], F32R)
                for pr in range(NPAIR):
                    pv0 = pv_ps.tile([128, 512], F32, tag="pv")
                    pv1 = pv_ps.tile([128, 512], F32, tag="pv")
                    for kip in range(0, nki, 2):
                        sc0 = score_ps.tile([128, 1024], F32, tag="sc")
                        sc1 = score_ps.tile([128, 1024], F32, tag="sc")
                        for u in range(2):
                            ki = kip + u
                            ksl = slice(128 * ki, 128 * (ki + 1))
                            usl = slice(512 * u, 512 * (u + 1))
                            nc.tensor.matmul(
                                sc0[:, usl],
                                kT_sb[0:64, pr, ksl],
                                qT_sb[0:64, pr, qsl],
                                start=True,
                                stop=True,
                                tile_position=(0, 0),
                            )
                            nc.tensor.matmul(
                                sc1[:, usl],
                                kT_sb[64:128, pr, ksl],
                                qT_sb[64:128, pr, qsl],
                                start=True,
                                stop=True,
                                tile_position=(64, 0),
                            )
                        p0 = p_pool.tile([128, 1024], F32R, tag="p")
                        p1 = p_pool.tile([128, 1024], F32R, tag="p")
                        nc.scalar.activation(
                            p0, sc0, mybir.ActivationFunctionType.Exp, scale=0.125
                        )
                        nc.scalar.activation(
                            p1, sc1, mybir.ActivationFunctionType.Exp, scale=0.125
                        )
                        for u in range(2):
                            ki = kip + u
                            off = 128 * (ki - 4 * t)
                            if off >= 0:  # diagonal tile: causal mask
                                msl = slice(512 * u + off, 512 * u + off + 128)
                                nc.vector.tensor_mul(p0[:, msl], p0[:, msl], tri_sb)
                                nc.vector.tensor_mul(p1[:, msl], p1[:, msl], tri_sb)
                            off2 = max(0, off)
                            psl = slice(512 * u + off2, 512 * (u + 1))
                            osl = slice(off2, 512)
                            nc.tensor.matmul(
                                pv0[0:65, osl],
                                v_sb[:, ki, 2 * pr, 0:65],
                                p0[:, psl],
                                start=(ki == 0),
                                stop=(ki == nki - 1),
                            )
                            nc.tensor.matmul(
                                pv1[0:65, osl],
                                v_sb[:, ki, 2 * pr + 1, 0:65],
                                p1[:, psl],
                                start=(ki == 0),
                                stop=(ki == nki - 1),
                            )
                        if debug and t == 0 and pr == 0 and kip == 0:
                            nc.sync.dma_start(out=dbg["p0"], in_=p0.bitcast(F32))
                            nc.sync.dma_start(out=dbg["p1"], in_=p1.bitcast(F32))
                    if debug and t == 0 and pr == 0:
                        pvc = z_pool.tile([65, 512], F32, tag="pvdbg")
                        nc.vector.tensor_copy(pvc, pv0[0:65, :])
                        nc.sync.dma_start(out=dbg["pv0"], in_=pvc)
                    # normalize each head by its denominator (row 64)
                    for h, pv in ((2 * pr, pv0), (2 * pr + 1, pv1)):
                        rl = rl_pool.tile([128, 512], F32, tag="rl")
                        # cross-partition write: denominator lives on PSUM
                        # partition 64; HW partition_broadcast reads its
                        # input from partition 0, so land the reciprocal there
                        nc.vector.reciprocal(rl[0:1, :], pv[64:65, :])
                        rlb = rlb_pool.tile([64, 512], F32, tag="rlb")
                        nc.gpsimd.partition_broadcast(rlb, rl[0:1, :])
                        if debug and t == 0 and h == 0:
                            nc.sync.dma_start(out=dbg["rlb"], in_=rlb)
                        nc.vector.tensor_mul(o_nt[:, h, :], pv[0:64, :], rlb)
                if debug and t == 0:
                    nc.sync.dma_start(out=dbg["ont"], in_=o_nt.bitcast(F32))
                # output projection for this q-tile (contract per head, K=64)
                for qs in range(4):
                    z_sb = z_pool.tile([128, D], F32)
                    for do_ in range(2):
                        zp = z_ps.tile([128, 512], F32)
                        for h in range(HPC):
                            nc.tensor.matmul(
                                zp,
                                o_nt[:, h, 128 * qs : 128 * (qs + 1)],
                                woT_sb[:, h, 512 * do_ : 512 * (do_ + 1)],
                                start=(h == 0),
                                stop=(h == HPC - 1),
                            )
                        nc.vector.tensor_copy(z_sb[:, 512 * do_ : 512 * (do_ + 1)], zp)
                    r0 = 512 * t + 128 * qs
                    nc.sync.dma_start(out=z[r0 : r0 + 128, :], in_=z_sb)


def _get_nc(debug=False):
    if debug not in _NC_CACHE:
        nc = bacc.Bacc(
            "TRN2", target_bir_lowering=False, debug=False, num_devices=NCORES
        )
        with tile.TileContext(nc) as tc:
            _emit(tc, debug=debug)
        nc.compile()
        _NC_CACHE[debug] = nc
    return _NC_CACHE[debug]


def _shard(inputs):
    def get(*names):
        for n in names:
            if n in inputs:
                return np.asarray(inputs[n], dtype=np.float32)
        raise KeyError(names)

    query = get("query")
    key_ = get("key_", "key")
    value = get("value")
    Wq, Wk, Wv, Wo = get("Wq"), get("Wk"), get("Wv"), get("Wo")
    bq, bk, bv = get("bq"), get("bk"), get("bv")
    tri = np.triu(np.ones((128, 128), dtype=np.float32))

    in_maps = []
    for c in range(NCORES):
        b, hg = c // 2, c % 2
        sl = slice(DPC * hg, DPC * (hg + 1))
        in_maps.append(
            {
                "xqT": np.ascontiguousarray(query[b].T),
                "xkT": np.ascontiguousarray(key_[b].T),
                "xvT": np.ascontiguousarray(value[b].T),
                "wqT": np.ascontiguousarray(Wq[sl].T),
                "wkT": np.ascontiguousarray(Wk[sl].T),
                "wvT": np.ascontiguousarray(Wv[sl].T),
                "woT": np.ascontiguousarray(Wo[:, sl].T),
                "bq": np.ascontiguousarray(bq[sl]),
                "bk": np.ascontiguousarray(bk[sl]),
                "bv": np.ascontiguousarray(bv[sl]),
                "tri": tri,
                "ones": np.ones((KT, HPC), dtype=np.float32),
            }
        )
    return in_maps


def _run(in_maps, trace=False, debug=False, **kwargs):
    nc = _get_nc(debug=debug)
    return bass_utils.run_bass_kernel_spmd(
        nc, in_maps, core_ids=list(range(len(in_maps))), trace=trace, **kwargs
    )


def _gather(results, inputs):
    bo = np.asarray(inputs["bo"], dtype=np.float32) if "bo" in inputs else 0.0
    out = np.empty((B, S, D), dtype=np.float32)
    for b in range(B):
        out[b] = results[2 * b]["z"] + results[2 * b + 1]["z"] + bo
    return out


def kernel(**inputs):
    in_maps = _shard(inputs)
    res = _run(in_maps)
    return _gather(res.results, inputs)


# revision 14
# speedup vs baseline: 6.3947x; 6.3947x over previous
"""Multi-head attention (B=4, S=2048, D=1024, H=16, causal) on 8 Trainium2 cores.

Sharding: core c -> (batch b = c//2, head-group hg = c%2, 8 heads each).
Each core computes its 8 heads' attention for its batch element plus the
partial output projection against the corresponding 512 columns of Wo.
Host sums the two partial projections per batch element and adds bo.

Device-side layouts (prepared on host as part of sharding):
  xqT/xkT/xvT [D=1024, S=2048]  -- x.T so the contraction dim (d) sits on
                                   SBUF partitions for all projection matmuls
  wqT/wkT/wvT [1024, 512]       -- W_part.T ([d, d'])
  woT [512, 1024]               -- Wo[:, part].T ([d', dout])
  bq/bk/bv [512], tri [128,128] -- triu(ones): tri[k,q] = 1 iff k <= q

All matmuls run as float32r (full fp32 storage, reduced-precision multiply,
1 cycle/row for moving free dim >= 256). Scores are computed transposed
(S_T[k, q]) so softmax needs no on-chip transposes: exp(s/8) on ScalarE
(no max subtraction; scores are ~N(0,1) for this problem's inputs), the
softmax denominator comes from a ones-column appended to V, and the
normalization happens on the [65, q] PV accumulator where l is a single
partition row.
"""

import os
import sys

import numpy as np

for _p in ("/opt/trn_rl_repo", "/root/.axon_site/_ro/trn_rl_repo"):
    if os.path.isdir(_p):
        if _p not in sys.path:
            sys.path.insert(0, _p)
        break

import concourse.bass as bass
import concourse.bacc as bacc
import concourse.tile as tile
from concourse import mybir
from concourse import bass_utils

B, S, D, H = 4, 2048, 1024, 16
HD = D // H            # 64
NCORES = 8
HPC = 8                # heads per core
DPC = 512              # d' (head dims) per core
NPAIR = 4              # head pairs per core
KT = S // 128          # 16 k-tiles
QT = S // 512          # 4 q-tiles (512 wide)
DT = D // 128          # 8 d-tiles
JT = DPC // 128        # 4 d'-tiles

F32 = mybir.dt.float32
F32R = mybir.dt.float32r

_NC_CACHE = {}


def _emit(tc, debug=False, reps=1):
    nc = tc.nc

    xqT = nc.dram_tensor("xqT", [D, S], F32R, kind="ExternalInput").ap()
    xkT = nc.dram_tensor("xkT", [D, S], F32R, kind="ExternalInput").ap()
    xvT = nc.dram_tensor("xvT", [D, S], F32R, kind="ExternalInput").ap()
    wqT = nc.dram_tensor("wqT", [D, DPC], F32R, kind="ExternalInput").ap()
    wkT = nc.dram_tensor("wkT", [D, DPC], F32R, kind="ExternalInput").ap()
    wvT = nc.dram_tensor("wvT", [D, DPC], F32R, kind="ExternalInput").ap()
    woT = nc.dram_tensor("woT", [DPC, D], F32R, kind="ExternalInput").ap()
    bqd = nc.dram_tensor("bq", [DPC], F32, kind="ExternalInput").ap()
    bkd = nc.dram_tensor("bk", [DPC], F32, kind="ExternalInput").ap()
    bvd = nc.dram_tensor("bv", [DPC], F32, kind="ExternalInput").ap()
    trid = nc.dram_tensor("tri", [128, 128], F32R, kind="ExternalInput").ap()
    onesd = nc.dram_tensor("ones", [KT, HPC], F32R, kind="ExternalInput").ap()
    z = nc.dram_tensor("z", [S, D], F32, kind="ExternalOutput").ap()
    dbg = {}
    if debug:
        dbg["qT"] = nc.dram_tensor("dbg_qT", [128, NPAIR, S], F32, kind="ExternalOutput").ap()
        dbg["kT"] = nc.dram_tensor("dbg_kT", [128, NPAIR, S], F32, kind="ExternalOutput").ap()
        dbg["v"] = nc.dram_tensor("dbg_v", [128, KT, HPC, 65], F32, kind="ExternalOutput").ap()
        dbg["p0"] = nc.dram_tensor("dbg_p0", [128, 1024], F32, kind="ExternalOutput").ap()
        dbg["p1"] = nc.dram_tensor("dbg_p1", [128, 1024], F32, kind="ExternalOutput").ap()
        dbg["pv0"] = nc.dram_tensor("dbg_pv0", [65, 512], F32, kind="ExternalOutput").ap()
        dbg["rlb"] = nc.dram_tensor("dbg_rlb", [64, 512], F32, kind="ExternalOutput").ap()
        dbg["ont"] = nc.dram_tensor("dbg_ont", [64, HPC, 512], F32, kind="ExternalOutput").ap()

    from contextlib import ExitStack

    for _rep in range(reps):
      with ExitStack() as stack:
        singles = stack.enter_context(tc.tile_pool(name="singles", bufs=1))
        qkv = stack.enter_context(tc.tile_pool(name="qkv", bufs=1))

        tri_sb = singles.tile([128, 128], F32R)
        nc.sync.dma_start(out=tri_sb, in_=trid)
        bvb = singles.tile([128, DPC], F32)
        nc.gpsimd.dma_start(out=bvb, in_=bvd.partition_broadcast(128))
        bq_sb = singles.tile([128, JT], F32)
        nc.sync.dma_start(out=bq_sb, in_=bqd.rearrange("(j p) -> p j", p=128))
        bk_sb = singles.tile([128, JT], F32)
        nc.sync.dma_start(out=bk_sb, in_=bkd.rearrange("(j p) -> p j", p=128))

        qT_sb = qkv.tile([128, NPAIR, S], F32R)   # [d'-in-pair, pair, q]
        kT_sb = qkv.tile([128, NPAIR, S], F32R)
        # V augmented per head: cols 0:64 = V_h, col 64 = ones (softmax denom)
        v_sb = qkv.tile([128, KT, HPC, 65], F32R)

        # ones column: v_ones = tri_view * 0 + 1 (memset can't write f32r)
        tri_view = tri_sb.rearrange("p (a b) -> p a b", a=KT).unsqueeze(3)
        nc.vector.tensor_scalar(
            v_sb[:, :, :, 64:65],
            tri_view,
            0.0,
            1.0,
            mybir.AluOpType.mult,
            mybir.AluOpType.add,
        )

        # ---------------- Phase 1a: Q and K projections -------------------
        with (
            tc.tile_pool(name="wqk", bufs=1) as wqk_pool,
            tc.tile_pool(name="xchunk", bufs=16) as xchunk_pool,
            tc.tile_pool(name="proj_ps", bufs=4, space="PSUM") as proj_ps,
        ):
            wq_sb = wqk_pool.tile([128, DT, DPC], F32R)
            nc.sync.dma_start(out=wq_sb, in_=wqT.rearrange("(dt p) c -> p dt c", p=128))
            wk_sb = wqk_pool.tile([128, DT, DPC], F32R)
            nc.sync.dma_start(out=wk_sb, in_=wkT.rearrange("(dt p) c -> p dt c", p=128))

            for w_sb, xT, dst_sb, b_sb in (
                (wq_sb, xqT, qT_sb, bq_sb),
                (wk_sb, xkT, kT_sb, bk_sb),
            ):
                for t in range(QT):
                    chunks = []
                    for dt in range(DT):
                        ch = xchunk_pool.tile([128, 512], F32R, tag="xch")
                        nc.sync.dma_start(
                            out=ch,
                            in_=xT[128 * dt : 128 * (dt + 1), 512 * t : 512 * (t + 1)],
                        )
                        chunks.append(ch)
                    for j in range(JT):
                        ps = proj_ps.tile([128, 512], F32)
                        for dt in range(DT):
                            nc.tensor.matmul(
                                ps,
                                w_sb[:, dt, 128 * j : 128 * (j + 1)],
                                chunks[dt],
                                start=(dt == 0),
                                stop=(dt == DT - 1),
                            )
                        nc.vector.tensor_scalar_add(
                            dst_sb[:, j, 512 * t : 512 * (t + 1)], ps, b_sb[:, j : j + 1]
                        )

        # ---------------- Phase 1b: V projection --------------------------
        with (
            tc.tile_pool(name="wv", bufs=1) as wv_pool,
            tc.tile_pool(name="xvchunk", bufs=16) as xvchunk_pool,
            tc.tile_pool(name="projv_ps", bufs=4, space="PSUM") as projv_ps,
        ):
            wv_sb = wv_pool.tile([128, DT, DPC], F32R)
            nc.sync.dma_start(out=wv_sb, in_=wvT.rearrange("(dt p) c -> p dt c", p=128))

            for ktg in range(4):  # groups of 4 k-tiles
                vchunks = []
                for dt in range(DT):
                    ch = xvchunk_pool.tile([128, 512], F32R, tag="xvch")
                    nc.sync.dma_start(
                        out=ch,
                        in_=xvT[
                            128 * dt : 128 * (dt + 1), 512 * ktg : 512 * (ktg + 1)
                        ],
                    )
                    vchunks.append(ch)
                for ksub in range(4):
                    kt = 4 * ktg + ksub
                    ps = projv_ps.tile([128, 512], F32)
                    for dt in range(DT):
                        nc.tensor.matmul(
                            ps,
                            vchunks[dt][:, 128 * ksub : 128 * (ksub + 1)],
                            wv_sb[:, dt, :],
                            start=(dt == 0),
                            stop=(dt == DT - 1),
                        )
                    # scatter heads into the augmented layout (+ bias)
                    ps4 = ps.rearrange("p (h c) -> p h c", h=HPC)
                    bv4 = bvb.rearrange("p (h c) -> p h c", h=HPC)
                    nc.vector.tensor_add(v_sb[:, kt, :, 0:64], ps4, bv4)

        if debug:
            nc.sync.dma_start(out=dbg["qT"], in_=qT_sb.bitcast(F32))
            nc.sync.dma_start(out=dbg["kT"], in_=kT_sb.bitcast(F32))
            nc.sync.dma_start(out=dbg["v"], in_=v_sb.bitcast(F32))

        # ---------------- Phase 2: attention + output projection ----------
        with (
            tc.tile_pool(name="wo", bufs=1) as wo_pool,
            tc.tile_pool(name="p_sb", bufs=4) as p_pool,
            tc.tile_pool(name="o_nt", bufs=2) as o_pool,
            tc.tile_pool(name="z_sb", bufs=2) as z_pool,
            tc.tile_pool(name="rl", bufs=2) as rl_pool,
            tc.tile_pool(name="rlb", bufs=4) as rlb_pool,
            tc.tile_pool(name="score_ps", bufs=2, space="PSUM") as score_ps,
            tc.tile_pool(name="pv_ps", bufs=2, space="PSUM") as pv_ps,
            tc.tile_pool(name="z_ps", bufs=2, space="PSUM") as z_ps,
        ):
            # per-head Wo.T rows, all at base partition 0: [64, head, dout]
            woT_sb = wo_pool.tile([64, HPC, D], F32R)
            nc.sync.dma_start(
                out=woT_sb, in_=woT.rearrange("(h p) c -> p h c", p=64)
            )

            for t in range(QT):
                nki = 4 * (t + 1)
                qsl = slice(512 * t, 512 * (t + 1))
                o_nt = o_pool.tile([64, HPC, 512], F32R)
                for pr in range(NPAIR):
                    pv0 = pv_ps.tile([128, 512], F32, tag="pv")
                    pv1 = pv_ps.tile([128, 512], F32, tag="pv")
                    for kip in range(0, nki, 2):
                        sc0 = score_ps.tile([128, 1024], F32, tag="sc")
                        sc1 = score_ps.tile([128, 1024], F32, tag="sc")
                        for u in range(2):
                            ki = kip + u
                            ksl = slice(128 * ki, 128 * (ki + 1))
                            usl = slice(512 * u, 512 * (u + 1))
                            nc.tensor.matmul(
                                sc0[:, usl],
                                kT_sb[0:64, pr, ksl],
                                qT_sb[0:64, pr, qsl],
                                start=True,
                                stop=True,
                                tile_position=(0, 0),
                            )
                            nc.tensor.matmul(
                                sc1[:, usl],
                                kT_sb[64:128, pr, ksl],
                                qT_sb[64:128, pr, qsl],
                                start=True,
                                stop=True,
                                tile_position=(64, 0),
                            )
                        p0 = p_pool.tile([128, 1024], F32R, tag="p")
                        p1 = p_pool.tile([128, 1024], F32R, tag="p")
                        nc.scalar.activation(
                            p0, sc0, mybir.ActivationFunctionType.Exp, scale=0.125
                        )
                        nc.scalar.activation(
                            p1, sc1, mybir.ActivationFunctionType.Exp, scale=0.125
                        )
                        for u in range(2):
                            ki = kip + u
                            off = 128 * (ki - 4 * t)
                            if off >= 0:  # diagonal tile: causal mask
                                msl = slice(512 * u + off, 512 * u + off + 128)
                                nc.vector.tensor_mul(p0[:, msl], p0[:, msl], tri_sb)
                                nc.vector.tensor_mul(p1[:, msl], p1[:, msl], tri_sb)
                            off2 = max(0, off)
                            psl = slice(512 * u + off2, 512 * (u + 1))
                            osl = slice(off2, 512)
                            nc.tensor.matmul(
                                pv0[0:65, osl],
                                v_sb[:, ki, 2 * pr, 0:65],
                                p0[:, psl],
                                start=(ki == 0),
                                stop=(ki == nki - 1),
                            )
                            nc.tensor.matmul(
                                pv1[0:65, osl],
                                v_sb[:, ki, 2 * pr + 1, 0:65],
                                p1[:, psl],
                                start=(ki == 0),
                                stop=(ki == nki - 1),
                            )
                        if debug and t == 0 and pr == 0 and kip == 0:
                            nc.sync.dma_start(out=dbg["p0"], in_=p0.bitcast(F32))
                            nc.sync.dma_start(out=dbg["p1"], in_=p1.bitcast(F32))
                    if debug and t == 0 and pr == 0:
                        pvc = z_pool.tile([65, 512], F32, tag="pvdbg")
                        nc.vector.tensor_copy(pvc, pv0[0:65, :])
                        nc.sync.dma_start(out=dbg["pv0"], in_=pvc)
                    # normalize each head by its denominator (row 64)
                    for h, pv in ((2 * pr, pv0), (2 * pr + 1, pv1)):
                        rl = rl_pool.tile([128, 512], F32, tag="rl")
                        # cross-partition write: denominator lives on PSUM
                        # partition 64; HW partition_broadcast reads its
                        # input from partition 0, so land the reciprocal there
                        nc.vector.reciprocal(rl[0:1, :], pv[64:65, :])
                        rlb = rlb_pool.tile([64, 512], F32, tag="rlb")
                        nc.gpsimd.partition_broadcast(rlb, rl[0:1, :])
                        if debug and t == 0 and h == 0:
                            nc.sync.dma_start(out=dbg["rlb"], in_=rlb)
                        nc.vector.tensor_mul(o_nt[:, h, :], pv[0:64, :], rlb)
                if debug and t == 0:
                    nc.sync.dma_start(out=dbg["ont"], in_=o_nt.bitcast(F32))
                # output projection for this q-tile (contract per head, K=64)
                for qs in range(4):
                    z_sb = z_pool.tile([128, D], F32)
                    for do_ in range(2):
                        zp = z_ps.tile([128, 512], F32)
                        for h in range(HPC):
                            nc.tensor.matmul(
                                zp,
                                o_nt[:, h, 128 * qs : 128 * (qs + 1)],
                                woT_sb[:, h, 512 * do_ : 512 * (do_ + 1)],
                                start=(h == 0),
                                stop=(h == HPC - 1),
                            )
                        nc.vector.tensor_copy(z_sb[:, 512 * do_ : 512 * (do_ + 1)], zp)
                    r0 = 512 * t + 128 * qs
                    nc.sync.dma_start(out=z[r0 : r0 + 128, :], in_=z_sb)


def _get_nc(debug=False, reps=1):
    key = (debug, reps)
    if key not in _NC_CACHE:
        nc = bacc.Bacc(
            "TRN2", target_bir_lowering=False, debug=False, num_devices=NCORES
        )
        with tile.TileContext(nc) as tc:
            _emit(tc, debug=debug, reps=reps)
        nc.compile()
        _NC_CACHE[key] = nc
    return _NC_CACHE[key]


def _shard(inputs):
    def get(*names):
        for n in names:
            if n in inputs:
                return np.asarray(inputs[n], dtype=np.float32)
        raise KeyError(names)

    query = get("query")
    key_ = get("key_", "key")
    value = get("value")
    Wq, Wk, Wv, Wo = get("Wq"), get("Wk"), get("Wv"), get("Wo")
    bq, bk, bv = get("bq"), get("bk"), get("bv")
    tri = np.triu(np.ones((128, 128), dtype=np.float32))

    in_maps = []
    for c in range(NCORES):
        b, hg = c // 2, c % 2
        sl = slice(DPC * hg, DPC * (hg + 1))
        in_maps.append(
            {
                "xqT": np.ascontiguousarray(query[b].T),
                "xkT": np.ascontiguousarray(key_[b].T),
                "xvT": np.ascontiguousarray(value[b].T),
                "wqT": np.ascontiguousarray(Wq[sl].T),
                "wkT": np.ascontiguousarray(Wk[sl].T),
                "wvT": np.ascontiguousarray(Wv[sl].T),
                "woT": np.ascontiguousarray(Wo[:, sl].T),
                "bq": np.ascontiguousarray(bq[sl]),
                "bk": np.ascontiguousarray(bk[sl]),
                "bv": np.ascontiguousarray(bv[sl]),
                "tri": tri,
                "ones": np.ones((KT, HPC), dtype=np.float32),
            }
        )
    return in_maps


def _run(in_maps, trace=False, debug=False, **kwargs):
    nc = _get_nc(debug=debug)
    return bass_utils.run_bass_kernel_spmd(
        nc, in_maps, core_ids=list(range(len(in_maps))), trace=trace, **kwargs
    )


def _gather(results, inputs):
    bo = np.asarray(inputs["bo"], dtype=np.float32) if "bo" in inputs else 0.0
    out = np.empty((B, S, D), dtype=np.float32)
    for b in range(B):
        out[b] = results[2 * b]["z"] + results[2 * b + 1]["z"] + bo
    return out


def kernel(**inputs):
    in_maps = _shard(inputs)
    res = _run(in_maps)
    return _gather(res.results, inputs)


# revision 20
# speedup vs baseline: 14.5236x; 2.2712x over previous
"""Multi-head attention (B=4, S=2048, D=1024, H=16, causal) on 8 Trainium2 cores.

Sharding: core c -> (batch b = c//2, head-group hg = c%2, 8 heads each).
Each core computes its 8 heads' attention for its batch element plus the
partial output projection against the corresponding 512 columns of Wo.
Host sums the two partial projections per batch element and adds bo.

Device-side layouts (prepared on host as part of sharding):
  xqT/xkT/xvT [D=1024, S=2048]  -- x.T so the contraction dim (d) sits on
                                   SBUF partitions for all projection matmuls
  wqT/wkT/wvT [1024, 512]       -- W_part.T ([d, d'])
  woT [512, 1024]               -- Wo[:, part].T ([d', dout])
  bq/bk/bv [512], tri [128,128] -- triu(ones): tri[k,q] = 1 iff k <= q

All matmuls run as float32r (full fp32 storage, reduced-precision multiply,
1 cycle/row for moving free dim >= 256). Scores are computed transposed
(S_T[k, q]) so softmax needs no on-chip transposes: exp(s/8) on ScalarE
(no max subtraction; scores are ~N(0,1) for this problem's inputs), the
softmax denominator comes from a ones-column appended to V, and the
normalization happens on the [65, q] PV accumulator where l is a single
partition row.
"""

import os
import sys

import numpy as np

for _p in ("/opt/trn_rl_repo", "/root/.axon_site/_ro/trn_rl_repo"):
    if os.path.isdir(_p):
        if _p not in sys.path:
            sys.path.insert(0, _p)
        break

import concourse.bass as bass
import concourse.bacc as bacc
import concourse.tile as tile
from concourse import mybir
from concourse import bass_utils

B, S, D, H = 4, 2048, 1024, 16
HD = D // H            # 64
NCORES = 8
HPC = 8                # heads per core
DPC = 512              # d' (head dims) per core
NPAIR = 4              # head pairs per core
KT = S // 128          # 16 k-tiles
QT = S // 512          # 4 q-tiles (512 wide)
DT = D // 128          # 8 d-tiles
JT = DPC // 128        # 4 d'-tiles

F32 = mybir.dt.float32
F32R = mybir.dt.float32r

_NC_CACHE = {}


def _emit(tc, debug=False, reps=1):
    nc = tc.nc

    xqT = nc.dram_tensor("xqT", [D, S], F32R, kind="ExternalInput").ap()
    xkT = nc.dram_tensor("xkT", [D, S], F32R, kind="ExternalInput").ap()
    xvT = nc.dram_tensor("xvT", [D, S], F32R, kind="ExternalInput").ap()
    wqT = nc.dram_tensor("wqT", [D, DPC], F32R, kind="ExternalInput").ap()
    wkT = nc.dram_tensor("wkT", [D, DPC], F32R, kind="ExternalInput").ap()
    wvT = nc.dram_tensor("wvT", [D, DPC], F32R, kind="ExternalInput").ap()
    woT = nc.dram_tensor("woT", [DPC, D], F32R, kind="ExternalInput").ap()
    bqd = nc.dram_tensor("bq", [DPC], F32, kind="ExternalInput").ap()
    bkd = nc.dram_tensor("bk", [DPC], F32, kind="ExternalInput").ap()
    bvd = nc.dram_tensor("bv", [DPC], F32, kind="ExternalInput").ap()
    trid = nc.dram_tensor("tri", [128, 128], F32R, kind="ExternalInput").ap()
    onesd = nc.dram_tensor("ones", [KT, HPC], F32R, kind="ExternalInput").ap()
    z = nc.dram_tensor("z", [S, D], F32, kind="ExternalOutput").ap()
    dbg = {}
    if debug:
        dbg["qT"] = nc.dram_tensor("dbg_qT", [128, NPAIR, S], F32, kind="ExternalOutput").ap()
        dbg["kT"] = nc.dram_tensor("dbg_kT", [128, NPAIR, S], F32, kind="ExternalOutput").ap()
        dbg["v"] = nc.dram_tensor("dbg_v", [128, KT, HPC, 65], F32, kind="ExternalOutput").ap()
        dbg["p0"] = nc.dram_tensor("dbg_p0", [128, 1024], F32, kind="ExternalOutput").ap()
        dbg["p1"] = nc.dram_tensor("dbg_p1", [128, 1024], F32, kind="ExternalOutput").ap()
        dbg["pv0"] = nc.dram_tensor("dbg_pv0", [65, 512], F32, kind="ExternalOutput").ap()
        dbg["rlb"] = nc.dram_tensor("dbg_rlb", [64, 512], F32, kind="ExternalOutput").ap()
        dbg["ont"] = nc.dram_tensor("dbg_ont", [128, NPAIR, 512], F32, kind="ExternalOutput").ap()

    from contextlib import ExitStack

    for _rep in range(reps):
      with ExitStack() as stack:
        singles = stack.enter_context(tc.tile_pool(name="singles", bufs=1))
        qkv = stack.enter_context(tc.tile_pool(name="qkv", bufs=1))

        tri_sb = singles.tile([128, 128], F32R)
        nc.sync.dma_start(out=tri_sb, in_=trid)
        bvb = singles.tile([128, DPC], F32)
        nc.gpsimd.dma_start(out=bvb, in_=bvd.partition_broadcast(128))
        bq_sb = singles.tile([128, JT], F32)
        nc.sync.dma_start(out=bq_sb, in_=bqd.rearrange("(j p) -> p j", p=128))
        bk_sb = singles.tile([128, JT], F32)
        nc.sync.dma_start(out=bk_sb, in_=bkd.rearrange("(j p) -> p j", p=128))

        qT_sb = qkv.tile([128, NPAIR, S], F32R)   # [d'-in-pair, pair, q]
        kT_sb = qkv.tile([128, NPAIR, S], F32R)
        # V augmented per head: cols 0:64 = V_h, col 64 = ones (softmax denom)
        v_sb = qkv.tile([128, KT, HPC, 65], F32R)

        # ones column: v_ones = tri_view * 0 + 1 (memset can't write f32r)
        tri_view = tri_sb.rearrange("p (a b) -> p a b", a=KT).unsqueeze(3)
        nc.vector.tensor_scalar(
            v_sb[:, :, :, 64:65],
            tri_view,
            0.0,
            1.0,
            mybir.AluOpType.mult,
            mybir.AluOpType.add,
        )

        # ---------------- Phase A: K projection (all k-slices) ------------
        with (
            tc.tile_pool(name="wk", bufs=1) as wk_pool,
            tc.tile_pool(name="xkchunk", bufs=8) as xk_pool,
            tc.tile_pool(name="kproj_ps", bufs=4, space="PSUM") as kproj_ps,
        ):
            wk_sb = wk_pool.tile([128, DT, DPC], F32R)

            # exp table warmup on ScalarE (off critical path; ~2.7us)
            wrm = wk_pool.tile([1, 1], F32)
            nc.scalar.activation(
                wrm, tri_sb[0:1, 0:1].bitcast(F32),
                mybir.ActivationFunctionType.Exp,
            )

            chunks = []
            for dt in range(DT):
                nc.sync.dma_start(
                    out=wk_sb[:, dt, :], in_=wkT[128 * dt : 128 * (dt + 1), :]
                )
                ch = xk_pool.tile([128, S], F32R, tag="xkch")
                nc.sync.dma_start(
                    out=ch, in_=xkT[128 * dt : 128 * (dt + 1), :]
                )
                chunks.append(ch)
            for t in range(QT):
                for j in range(JT):
                    ps = kproj_ps.tile([128, 512], F32)
                    for dt in range(DT):
                        nc.tensor.matmul(
                            ps,
                            wk_sb[:, dt, 128 * j : 128 * (j + 1)],
                            chunks[dt][:, 512 * t : 512 * (t + 1)],
                            start=(dt == 0),
                            stop=(dt == DT - 1),
                        )
                    nc.vector.tensor_scalar_add(
                        kT_sb[:, j, 512 * t : 512 * (t + 1)], ps, bk_sb[:, j : j + 1]
                    )

        # ---- Phase B: per q-tile {V-proj, Q-proj, attention, out-proj} ----
        # PE-dense projection work overlaps the ACT-bound attention stretches;
        # DMAs are emitted just-in-time (HWDGE FIFO is in-order).
        with (
            tc.tile_pool(name="wqv", bufs=1) as wqv_pool,
            tc.tile_pool(name="xc2", bufs=8) as xc2_pool,
            tc.tile_pool(name="wo", bufs=1) as wo_pool,
            tc.tile_pool(name="p_sb", bufs=4) as p_pool,
            tc.tile_pool(name="o_nt", bufs=1) as o_pool,
            tc.tile_pool(name="z_sb", bufs=2) as z_pool,
            tc.tile_pool(name="rl", bufs=2) as rl_pool,
            tc.tile_pool(name="rlb", bufs=2) as rlb_pool,
            tc.tile_pool(name="score_ps", bufs=2, space="PSUM") as score_ps,
            tc.tile_pool(name="pv_ps", bufs=2, space="PSUM") as pv_ps,
            tc.tile_pool(name="proj_ps", bufs=1, space="PSUM") as proj_ps,
            tc.tile_pool(name="z_ps", bufs=1, space="PSUM") as z_ps,
        ):
            wq_sb = wqv_pool.tile([128, DT, DPC], F32R)
            wv_sb = wqv_pool.tile([128, DT, DPC], F32R)
            woT_sb = wo_pool.tile([128, JT, D], F32R)

            for t in range(QT):
                # ---- V projection for k-tiles 4t..4t+3 ----
                chv = []
                for dt in range(DT):
                    if t == 0:
                        nc.sync.dma_start(
                            out=wv_sb[:, dt, :],
                            in_=wvT[128 * dt : 128 * (dt + 1), :],
                        )
                    ch = xc2_pool.tile([128, 512], F32R, tag="xc2")
                    nc.sync.dma_start(
                        out=ch,
                        in_=xvT[128 * dt : 128 * (dt + 1), 512 * t : 512 * (t + 1)],
                    )
                    chv.append(ch)
                for ksub in range(4):
                    kt = 4 * t + ksub
                    ps = proj_ps.tile([128, 512], F32, tag="proj")
                    for dt in range(DT):
                        nc.tensor.matmul(
                            ps,
                            chv[dt][:, 128 * ksub : 128 * (ksub + 1)],
                            wv_sb[:, dt, :],
                            start=(dt == 0),
                            stop=(dt == DT - 1),
                        )
                    ps4 = ps.rearrange("p (h c) -> p h c", h=HPC)
                    bv4 = bvb.rearrange("p (h c) -> p h c", h=HPC)
                    nc.vector.tensor_add(v_sb[:, kt, :, 0:64], ps4, bv4)

                # ---- Q projection for q-slice t ----
                chq = []
                for dt in range(DT):
                    if t == 0:
                        nc.sync.dma_start(
                            out=wq_sb[:, dt, :],
                            in_=wqT[128 * dt : 128 * (dt + 1), :],
                        )
                    ch = xc2_pool.tile([128, 512], F32R, tag="xc2")
                    nc.sync.dma_start(
                        out=ch,
                        in_=xqT[128 * dt : 128 * (dt + 1), 512 * t : 512 * (t + 1)],
                    )
                    chq.append(ch)
                if t == 0:
                    nc.sync.dma_start(
                        out=woT_sb, in_=woT.rearrange("(j p) c -> p j c", p=128)
                    )
                for j in range(JT):
                    ps = proj_ps.tile([128, 512], F32, tag="proj")
                    for dt in range(DT):
                        nc.tensor.matmul(
                            ps,
                            wq_sb[:, dt, 128 * j : 128 * (j + 1)],
                            chq[dt],
                            start=(dt == 0),
                            stop=(dt == DT - 1),
                        )
                    nc.vector.tensor_scalar_add(
                        qT_sb[:, j, 512 * t : 512 * (t + 1)], ps, bq_sb[:, j : j + 1]
                    )

                # ---- attention for q-tile t ----
                nki = 4 * (t + 1)
                qsl = slice(512 * t, 512 * (t + 1))
                o_nt = o_pool.tile([128, NPAIR, 512], F32R)
                for pr in range(NPAIR):
                    pv0 = pv_ps.tile([128, 512], F32, tag="pv")
                    pv1 = pv_ps.tile([128, 512], F32, tag="pv")
                    for kip in range(0, nki, 2):
                        sc0 = score_ps.tile([128, 1024], F32, tag="sc")
                        sc1 = score_ps.tile([128, 1024], F32, tag="sc")
                        for u in range(2):
                            ki = kip + u
                            ksl = slice(128 * ki, 128 * (ki + 1))
                            usl = slice(512 * u, 512 * (u + 1))
                            nc.tensor.matmul(
                                sc0[:, usl],
                                kT_sb[0:64, pr, ksl],
                                qT_sb[0:64, pr, qsl],
                                start=True,
                                stop=True,
                                tile_position=(0, 0),
                            )
                            nc.tensor.matmul(
                                sc1[:, usl],
                                kT_sb[64:128, pr, ksl],
                                qT_sb[64:128, pr, qsl],
                                start=True,
                                stop=True,
                                tile_position=(64, 0),
                            )
                        p0 = p_pool.tile([128, 1024], F32R, tag="p")
                        p1 = p_pool.tile([128, 1024], F32R, tag="p")
                        nc.scalar.activation(
                            p0, sc0, mybir.ActivationFunctionType.Exp, scale=0.125
                        )
                        nc.scalar.activation(
                            p1, sc1, mybir.ActivationFunctionType.Exp, scale=0.125
                        )
                        for u in range(2):
                            ki = kip + u
                            off = 128 * (ki - 4 * t)
                            if off >= 0:  # diagonal tile: causal mask
                                msl = slice(512 * u + off, 512 * u + off + 128)
                                nc.vector.tensor_mul(p0[:, msl], p0[:, msl], tri_sb)
                                nc.vector.tensor_mul(p1[:, msl], p1[:, msl], tri_sb)
                            off2 = max(0, off)
                            psl = slice(512 * u + off2, 512 * (u + 1))
                            osl = slice(off2, 512)
                            nc.tensor.matmul(
                                pv0[0:65, osl],
                                v_sb[:, ki, 2 * pr, 0:65],
                                p0[:, psl],
                                start=(ki == 0),
                                stop=(ki == nki - 1),
                            )
                            nc.tensor.matmul(
                                pv1[0:65, osl],
                                v_sb[:, ki, 2 * pr + 1, 0:65],
                                p1[:, psl],
                                start=(ki == 0),
                                stop=(ki == nki - 1),
                            )
                        if debug and t == 0 and pr == 0 and kip == 0:
                            nc.sync.dma_start(out=dbg["p0"], in_=p0.bitcast(F32))
                            nc.sync.dma_start(out=dbg["p1"], in_=p1.bitcast(F32))
                    if debug and t == 0 and pr == 0:
                        pvc = z_pool.tile([65, 512], F32, tag="pvdbg")
                        nc.vector.tensor_copy(pvc, pv0[0:65, :])
                        nc.sync.dma_start(out=dbg["pv0"], in_=pvc)
                    # normalize each head by its denominator (row 64)
                    for par, pv in ((0, pv0), (1, pv1)):
                        rl = rl_pool.tile([128, 512], F32, tag="rl")
                        # HW partition_broadcast reads its input from
                        # partition 0, so land the reciprocal there
                        nc.vector.reciprocal(rl[0:1, :], pv[64:65, :])
                        rlb = rlb_pool.tile([64, 512], F32, tag="rlb")
                        nc.gpsimd.partition_broadcast(rlb, rl[0:1, :])
                        if debug and t == 0 and pr == 0 and par == 0:
                            nc.sync.dma_start(out=dbg["rlb"], in_=rlb)
                        # even head -> partitions 0:64, odd head -> 64:128
                        # (cross-base DVE write for the odd half)
                        nc.vector.tensor_mul(
                            o_nt[64 * par : 64 * par + 64, pr, :],
                            pv[0:64, :],
                            rlb,
                        )
                if debug and t == 0:
                    nc.sync.dma_start(out=dbg["ont"], in_=o_nt.bitcast(F32))
                # ---- output projection for this q-tile (per pair, K=128) ----
                for qs in range(4):
                    z_sb = z_pool.tile([128, D], F32)
                    for do_ in range(2):
                        zp = z_ps.tile([128, 512], F32)
                        for j in range(JT):
                            nc.tensor.matmul(
                                zp,
                                o_nt[:, j, 128 * qs : 128 * (qs + 1)],
                                woT_sb[:, j, 512 * do_ : 512 * (do_ + 1)],
                                start=(j == 0),
                                stop=(j == JT - 1),
                            )
                        nc.vector.tensor_copy(z_sb[:, 512 * do_ : 512 * (do_ + 1)], zp)
                    r0 = 512 * t + 128 * qs
                    nc.sync.dma_start(out=z[r0 : r0 + 128, :], in_=z_sb)

        if debug:
            nc.sync.dma_start(out=dbg["qT"], in_=qT_sb.bitcast(F32))
            nc.sync.dma_start(out=dbg["kT"], in_=kT_sb.bitcast(F32))
            nc.sync.dma_start(out=dbg["v"], in_=v_sb.bitcast(F32))


def _get_nc(debug=False, reps=1):
    key = (debug, reps)
    if key not in _NC_CACHE:
        nc = bacc.Bacc(
            "TRN2", target_bir_lowering=False, debug=False, num_devices=NCORES
        )
        with tile.TileContext(nc) as tc:
            _emit(tc, debug=debug, reps=reps)
        nc.compile()
        _NC_CACHE[key] = nc
    return _NC_CACHE[key]


def _shard(inputs):
    def get(*names):
        for n in names:
            if n in inputs:
                return np.asarray(inputs[n], dtype=np.float32)
        raise KeyError(names)

    query = get("query")
    key_ = get("key_", "key")
    value = get("value")
    Wq, Wk, Wv, Wo = get("Wq"), get("Wk"), get("Wv"), get("Wo")
    bq, bk, bv = get("bq"), get("bk"), get("bv")
    tri = np.triu(np.ones((128, 128), dtype=np.float32))

    in_maps = []
    for c in range(NCORES):
        b, hg = c // 2, c % 2
        sl = slice(DPC * hg, DPC * (hg + 1))
        in_maps.append(
            {
                "xqT": np.ascontiguousarray(query[b].T),
                "xkT": np.ascontiguousarray(key_[b].T),
                "xvT": np.ascontiguousarray(value[b].T),
                "wqT": np.ascontiguousarray(Wq[sl].T),
                "wkT": np.ascontiguousarray(Wk[sl].T),
                "wvT": np.ascontiguousarray(Wv[sl].T),
                "woT": np.ascontiguousarray(Wo[:, sl].T),
                "bq": np.ascontiguousarray(bq[sl]),
                "bk": np.ascontiguousarray(bk[sl]),
                "bv": np.ascontiguousarray(bv[sl]),
                "tri": tri,
                "ones": np.ones((KT, HPC), dtype=np.float32),
            }
        )
    return in_maps


def _run(in_maps, trace=False, debug=False, **kwargs):
    nc = _get_nc(debug=debug)
    return bass_utils.run_bass_kernel_spmd(
        nc, in_maps, core_ids=list(range(len(in_maps))), trace=trace, **kwargs
    )


def _gather(results, inputs):
    bo = np.asarray(inputs["bo"], dtype=np.float32) if "bo" in inputs else 0.0
    out = np.empty((B, S, D), dtype=np.float32)
    for b in range(B):
        out[b] = results[2 * b]["z"] + results[2 * b + 1]["z"] + bo
    return out


def kernel(**inputs):
    in_maps = _shard(inputs)
    res = _run(in_maps)
    return _gather(res.results, inputs)


# revision 23
# speedup vs baseline: 20.8960x; 1.4388x over previous
"""Multi-head attention (B=4, S=2048, D=1024, H=16, causal) on 8 Trainium2 cores.

Sharding: core c -> (batch b = c//2, head-group hg = c%2, 8 heads each).
Each core computes its 8 heads' attention for its batch element plus the
partial output projection against the corresponding 512 columns of Wo.
Host sums the two partial projections per batch element and adds bo.

Device-side layouts (prepared on host as part of sharding):
  xqT/xkT/xvT [D=1024, S=2048]  -- x.T so the contraction dim (d) sits on
                                   SBUF partitions for all projection matmuls
  wqT/wkT/wvT [1024, 512]       -- W_part.T ([d, d'])
  woT [512, 1024]               -- Wo[:, part].T ([d', dout])
  bq/bk/bv [512], tri [128,128] -- triu(ones): tri[k,q] = 1 iff k <= q

All matmuls run as float32r (full fp32 storage, reduced-precision multiply,
1 cycle/row for moving free dim >= 256). Scores are computed transposed
(S_T[k, q]) so softmax needs no on-chip transposes: exp(s/8) on ScalarE
(no max subtraction; scores are ~N(0,1) for this problem's inputs), the
softmax denominator comes from a ones-column appended to V, and the
normalization happens on the [65, q] PV accumulator where l is a single
partition row.
"""

import os
import sys

import numpy as np

for _p in ("/opt/trn_rl_repo", "/root/.axon_site/_ro/trn_rl_repo"):
    if os.path.isdir(_p):
        if _p not in sys.path:
            sys.path.insert(0, _p)
        break

import concourse.bass as bass
import concourse.bacc as bacc
import concourse.tile as tile
from concourse import mybir
from concourse import bass_utils

B, S, D, H = 4, 2048, 1024, 16
HD = D // H            # 64
NCORES = 8
HPC = 8                # heads per core
DPC = 512              # d' (head dims) per core
NPAIR = 4              # head pairs per core
KT = S // 128          # 16 k-tiles
QT = S // 512          # 4 q-tiles (512 wide)
DT = D // 128          # 8 d-tiles
JT = DPC // 128        # 4 d'-tiles

F32 = mybir.dt.float32
F32R = mybir.dt.float32r

_NC_CACHE = {}


def _emit(tc, debug=False, reps=1):
    nc = tc.nc

    xqT = nc.dram_tensor("xqT", [D, S], F32R, kind="ExternalInput").ap()
    xkT = nc.dram_tensor("xkT", [D, S], F32R, kind="ExternalInput").ap()
    xvT = nc.dram_tensor("xvT", [D, S], F32R, kind="ExternalInput").ap()
    wqT = nc.dram_tensor("wqT", [D, DPC], F32R, kind="ExternalInput").ap()
    wkT = nc.dram_tensor("wkT", [D, DPC], F32R, kind="ExternalInput").ap()
    wvT = nc.dram_tensor("wvT", [D, DPC], F32R, kind="ExternalInput").ap()
    woT = nc.dram_tensor("woT", [DPC, D], F32R, kind="ExternalInput").ap()
    bqd = nc.dram_tensor("bq", [DPC], F32, kind="ExternalInput").ap()
    bkd = nc.dram_tensor("bk", [DPC], F32, kind="ExternalInput").ap()
    bvd = nc.dram_tensor("bv", [DPC], F32, kind="ExternalInput").ap()
    trid = nc.dram_tensor("tri", [128, 128], F32R, kind="ExternalInput").ap()
    onesd = nc.dram_tensor("ones", [KT, HPC], F32R, kind="ExternalInput").ap()
    z = nc.dram_tensor("z", [S, D], F32, kind="ExternalOutput").ap()
    dbg = {}
    if debug:
        dbg["qT"] = nc.dram_tensor("dbg_qT", [128, NPAIR, S], F32, kind="ExternalOutput").ap()
        dbg["kT"] = nc.dram_tensor("dbg_kT", [128, NPAIR, S], F32, kind="ExternalOutput").ap()
        dbg["v"] = nc.dram_tensor("dbg_v", [128, KT, HPC, 65], F32, kind="ExternalOutput").ap()
        dbg["p0"] = nc.dram_tensor("dbg_p0", [128, 1024], F32, kind="ExternalOutput").ap()
        dbg["p1"] = nc.dram_tensor("dbg_p1", [128, 1024], F32, kind="ExternalOutput").ap()
        dbg["pv0"] = nc.dram_tensor("dbg_pv0", [65, 512], F32, kind="ExternalOutput").ap()
        dbg["rlb"] = nc.dram_tensor("dbg_rlb", [64, 512], F32, kind="ExternalOutput").ap()
        dbg["ont"] = nc.dram_tensor("dbg_ont", [128, NPAIR, 512], F32, kind="ExternalOutput").ap()

    from contextlib import ExitStack

    for _rep in range(reps):
      with ExitStack() as stack:
        singles = stack.enter_context(tc.tile_pool(name="singles", bufs=1))
        qkv = stack.enter_context(tc.tile_pool(name="qkv", bufs=1))

        tri_sb = singles.tile([128, 128], F32R)
        nc.sync.dma_start(out=tri_sb, in_=trid)
        bvb = singles.tile([128, DPC], F32)
        nc.gpsimd.dma_start(out=bvb, in_=bvd.partition_broadcast(128))
        bq_sb = singles.tile([128, JT], F32)
        nc.sync.dma_start(out=bq_sb, in_=bqd.rearrange("(j p) -> p j", p=128))
        bk_sb = singles.tile([128, JT], F32)
        nc.sync.dma_start(out=bk_sb, in_=bkd.rearrange("(j p) -> p j", p=128))

        qT_sb = qkv.tile([128, NPAIR, S], F32R)   # [d'-in-pair, pair, q]
        kT_sb = qkv.tile([128, NPAIR, S], F32R)
        # V augmented per head: cols 0:64 = V_h, col 64 = ones (softmax denom)
        v_sb = qkv.tile([128, KT, HPC, 65], F32R)

        # ones column: v_ones = tri_view * 0 + 1 (memset can't write f32r)
        tri_view = tri_sb.rearrange("p (a b) -> p a b", a=KT).unsqueeze(3)
        nc.vector.tensor_scalar(
            v_sb[:, :, :, 64:65],
            tri_view,
            0.0,
            1.0,
            mybir.AluOpType.mult,
            mybir.AluOpType.add,
        )

        # -------- Phase A: all projections, order K -> Q(rev) -> V ---------
        # Per-512-column chunk loads keep the DMA->matmul pipeline fine-
        # grained; Q is projected t=3 first so attention (heaviest tile
        # first) starts as early as possible; V streams last -- PV consumes
        # V k-tile-by-k-tile in production order.
        with (
            tc.tile_pool(name="wslot", bufs=2) as w_pool,
            tc.tile_pool(name="xchunk", bufs=16) as x_pool,
            tc.tile_pool(name="proj_ps", bufs=4, space="PSUM") as proj_ps,
        ):
            # exp table warmup on ScalarE (off critical path; ~2.7us)
            wrm = w_pool.tile([1, 1], F32, tag="wrm")
            nc.scalar.activation(
                wrm, tri_sb[0:1, 0:1].bitcast(F32),
                mybir.ActivationFunctionType.Exp,
            )

            def load_w(wT):
                w_sb = w_pool.tile([128, DT, DPC], F32R, tag="w")
                for dt in range(DT):
                    nc.sync.dma_start(
                        out=w_sb[:, dt, :], in_=wT[128 * dt : 128 * (dt + 1), :]
                    )
                return w_sb

            def load_chunks(xT, t):
                chunks = []
                for dt in range(DT):
                    ch = x_pool.tile([128, 512], F32R, tag="xch")
                    nc.sync.dma_start(
                        out=ch,
                        in_=xT[128 * dt : 128 * (dt + 1), 512 * t : 512 * (t + 1)],
                    )
                    chunks.append(ch)
                return chunks

            # K projection (t ascending)
            w_sb = load_w(wkT)
            for t in range(QT):
                chunks = load_chunks(xkT, t)
                for j in range(JT):
                    ps = proj_ps.tile([128, 512], F32)
                    for dt in range(DT):
                        nc.tensor.matmul(
                            ps,
                            w_sb[:, dt, 128 * j : 128 * (j + 1)],
                            chunks[dt],
                            start=(dt == 0),
                            stop=(dt == DT - 1),
                        )
                    nc.vector.tensor_scalar_add(
                        kT_sb[:, j, 512 * t : 512 * (t + 1)], ps, bk_sb[:, j : j + 1]
                    )
            # Q projection, heavy-attention-first order (t = 3..0)
            w_sb = load_w(wqT)
            for t in reversed(range(QT)):
                chunks = load_chunks(xqT, t)
                for j in range(JT):
                    ps = proj_ps.tile([128, 512], F32)
                    for dt in range(DT):
                        nc.tensor.matmul(
                            ps,
                            w_sb[:, dt, 128 * j : 128 * (j + 1)],
                            chunks[dt],
                            start=(dt == 0),
                            stop=(dt == DT - 1),
                        )
                    nc.vector.tensor_scalar_add(
                        qT_sb[:, j, 512 * t : 512 * (t + 1)], ps, bq_sb[:, j : j + 1]
                    )
            # V projection (k-tiles ascending; stationary = x chunk slices)
            w_sb = load_w(wvT)
            for ktg in range(4):
                chunks = load_chunks(xvT, ktg)
                for ksub in range(4):
                    kt = 4 * ktg + ksub
                    ps = proj_ps.tile([128, 512], F32)
                    for dt in range(DT):
                        nc.tensor.matmul(
                            ps,
                            chunks[dt][:, 128 * ksub : 128 * (ksub + 1)],
                            w_sb[:, dt, :],
                            start=(dt == 0),
                            stop=(dt == DT - 1),
                        )
                    ps4 = ps.rearrange("p (h c) -> p h c", h=HPC)
                    bv4 = bvb.rearrange("p (h c) -> p h c", h=HPC)
                    nc.vector.tensor_add(v_sb[:, kt, :, 0:64], ps4, bv4)

        # -------- Phase B: attention + output projection, t = 3..0 ---------
        with (
            tc.tile_pool(name="wo", bufs=1) as wo_pool,
            tc.tile_pool(name="p_sb", bufs=4) as p_pool,
            tc.tile_pool(name="o_nt", bufs=2) as o_pool,
            tc.tile_pool(name="z_sb", bufs=2) as z_pool,
            tc.tile_pool(name="rl", bufs=2) as rl_pool,
            tc.tile_pool(name="rlb", bufs=2) as rlb_pool,
            tc.tile_pool(name="score_ps", bufs=2, space="PSUM") as score_ps,
            tc.tile_pool(name="pv_ps", bufs=2, space="PSUM") as pv_ps,
            tc.tile_pool(name="z_ps", bufs=2, space="PSUM") as z_ps,
        ):
            woT_sb = wo_pool.tile([128, JT, D], F32R)
            nc.sync.dma_start(
                out=woT_sb, in_=woT.rearrange("(j p) c -> p j c", p=128)
            )

            for t in reversed(range(QT)):
                nki = 4 * (t + 1)
                qsl = slice(512 * t, 512 * (t + 1))
                o_nt = o_pool.tile([128, NPAIR, 512], F32R)
                for pr in range(NPAIR):
                    pv0 = pv_ps.tile([128, 512], F32, tag="pv")
                    pv1 = pv_ps.tile([128, 512], F32, tag="pv")
                    for kip in range(0, nki, 2):
                        sc0 = score_ps.tile([128, 1024], F32, tag="sc")
                        sc1 = score_ps.tile([128, 1024], F32, tag="sc")
                        for u in range(2):
                            ki = kip + u
                            ksl = slice(128 * ki, 128 * (ki + 1))
                            usl = slice(512 * u, 512 * (u + 1))
                            nc.tensor.matmul(
                                sc0[:, usl],
                                kT_sb[0:64, pr, ksl],
                                qT_sb[0:64, pr, qsl],
                                start=True,
                                stop=True,
                                tile_position=(0, 0),
                            )
                            nc.tensor.matmul(
                                sc1[:, usl],
                                kT_sb[64:128, pr, ksl],
                                qT_sb[64:128, pr, qsl],
                                start=True,
                                stop=True,
                                tile_position=(64, 0),
                            )
                        p0 = p_pool.tile([128, 1024], F32R, tag="p")
                        p1 = p_pool.tile([128, 1024], F32R, tag="p")
                        nc.scalar.activation(
                            p0, sc0, mybir.ActivationFunctionType.Exp, scale=0.125
                        )
                        nc.scalar.activation(
                            p1, sc1, mybir.ActivationFunctionType.Exp, scale=0.125
                        )
                        for u in range(2):
                            ki = kip + u
                            off = 128 * (ki - 4 * t)
                            if off >= 0:  # diagonal tile: causal mask
                                msl = slice(512 * u + off, 512 * u + off + 128)
                                nc.vector.tensor_mul(p0[:, msl], p0[:, msl], tri_sb)
                                nc.vector.tensor_mul(p1[:, msl], p1[:, msl], tri_sb)
                            off2 = max(0, off)
                            psl = slice(512 * u + off2, 512 * (u + 1))
                            osl = slice(off2, 512)
                            nc.tensor.matmul(
                                pv0[0:65, osl],
                                v_sb[:, ki, 2 * pr, 0:65],
                                p0[:, psl],
                                start=(ki == 0),
                                stop=(ki == nki - 1),
                            )
                            nc.tensor.matmul(
                                pv1[0:65, osl],
                                v_sb[:, ki, 2 * pr + 1, 0:65],
                                p1[:, psl],
                                start=(ki == 0),
                                stop=(ki == nki - 1),
                            )
                        if debug and t == 0 and pr == 0 and kip == 0:
                            nc.sync.dma_start(out=dbg["p0"], in_=p0.bitcast(F32))
                            nc.sync.dma_start(out=dbg["p1"], in_=p1.bitcast(F32))
                    if debug and t == 0 and pr == 0:
                        pvc = z_pool.tile([65, 512], F32, tag="pvdbg")
                        nc.vector.tensor_copy(pvc, pv0[0:65, :])
                        nc.sync.dma_start(out=dbg["pv0"], in_=pvc)
                    # normalize each head by its denominator (row 64)
                    for par, pv in ((0, pv0), (1, pv1)):
                        rl = rl_pool.tile([128, 512], F32, tag="rl")
                        # HW partition_broadcast reads its input from
                        # partition 0, so land the reciprocal there
                        nc.vector.reciprocal(rl[0:1, :], pv[64:65, :])
                        rlb = rlb_pool.tile([64, 512], F32, tag="rlb")
                        nc.gpsimd.partition_broadcast(rlb, rl[0:1, :])
                        if debug and t == 0 and pr == 0 and par == 0:
                            nc.sync.dma_start(out=dbg["rlb"], in_=rlb)
                        # even head -> partitions 0:64, odd head -> 64:128
                        # (cross-base DVE write for the odd half)
                        nc.vector.tensor_mul(
                            o_nt[64 * par : 64 * par + 64, pr, :],
                            pv[0:64, :],
                            rlb,
                        )
                if debug and t == 0:
                    nc.sync.dma_start(out=dbg["ont"], in_=o_nt.bitcast(F32))
                # output projection for this q-tile (contract per pair, K=128)
                for qs in range(4):
                    z_sb = z_pool.tile([128, D], F32)
                    for do_ in range(2):
                        zp = z_ps.tile([128, 512], F32)
                        for j in range(JT):
                            nc.tensor.matmul(
                                zp,
                                o_nt[:, j, 128 * qs : 128 * (qs + 1)],
                                woT_sb[:, j, 512 * do_ : 512 * (do_ + 1)],
                                start=(j == 0),
                                stop=(j == JT - 1),
                            )
                        nc.vector.tensor_copy(z_sb[:, 512 * do_ : 512 * (do_ + 1)], zp)
                    r0 = 512 * t + 128 * qs
                    nc.sync.dma_start(out=z[r0 : r0 + 128, :], in_=z_sb)

        if debug:
            nc.sync.dma_start(out=dbg["qT"], in_=qT_sb.bitcast(F32))
            nc.sync.dma_start(out=dbg["kT"], in_=kT_sb.bitcast(F32))
            nc.sync.dma_start(out=dbg["v"], in_=v_sb.bitcast(F32))


def _get_nc(debug=False, reps=1):
    key = (debug, reps)
    if key not in _NC_CACHE:
        nc = bacc.Bacc(
            "TRN2", target_bir_lowering=False, debug=False, num_devices=NCORES
        )
        with tile.TileContext(nc) as tc:
            _emit(tc, debug=debug, reps=reps)
        nc.compile()
        _NC_CACHE[key] = nc
    return _NC_CACHE[key]


def _shard(inputs):
    def get(*names):
        for n in names:
            if n in inputs:
                return np.asarray(inputs[n], dtype=np.float32)
        raise KeyError(names)

    query = get("query")
    key_ = get("key_", "key")
    value = get("value")
    Wq, Wk, Wv, Wo = get("Wq"), get("Wk"), get("Wv"), get("Wo")
    bq, bk, bv = get("bq"), get("bk"), get("bv")
    tri = np.triu(np.ones((128, 128), dtype=np.float32))

    in_maps = []
    for c in range(NCORES):
        b, hg = c // 2, c % 2
        sl = slice(DPC * hg, DPC * (hg + 1))
        in_maps.append(
            {
                "xqT": np.ascontiguousarray(query[b].T),
                "xkT": np.ascontiguousarray(key_[b].T),
                "xvT": np.ascontiguousarray(value[b].T),
                "wqT": np.ascontiguousarray(Wq[sl].T),
                "wkT": np.ascontiguousarray(Wk[sl].T),
                "wvT": np.ascontiguousarray(Wv[sl].T),
                "woT": np.ascontiguousarray(Wo[:, sl].T),
                "bq": np.ascontiguousarray(bq[sl]),
                "bk": np.ascontiguousarray(bk[sl]),
                "bv": np.ascontiguousarray(bv[sl]),
                "tri": tri,
                "ones": np.ones((KT, HPC), dtype=np.float32),
            }
        )
    return in_maps


def _run(in_maps, trace=False, debug=False, **kwargs):
    nc = _get_nc(debug=debug)
    return bass_utils.run_bass_kernel_spmd(
        nc, in_maps, core_ids=list(range(len(in_maps))), trace=trace, **kwargs
    )


def _gather(results, inputs):
    bo = np.asarray(inputs["bo"], dtype=np.float32) if "bo" in inputs else 0.0
    out = np.empty((B, S, D), dtype=np.float32)
    for b in range(B):
        out[b] = results[2 * b]["z"] + results[2 * b + 1]["z"] + bo
    return out


def kernel(**inputs):
    in_maps = _shard(inputs)
    res = _run(in_maps)
    return _gather(res.results, inputs)
